# revision 1
# baseline (speedup 1.0000x reference)
"""Trainium2 Bass kernel for nn_Attention_49177375539262 (sparse_attention).

Math (per group g of b*h*B = 512 groups, L=256, D=64):
  sigma_q = q @ Wq^T + 1        [L]
  sigma_k = k @ Wk^T + 1        [L]
  sim     = q @ k^T             [L, L]
  sim2    = sim * outer(sigma_q, sigma_k)
  theta_i = (sim * (1-I)) @ W1 = q @ (k^T @ W1) - (q_i . k_i) * W1_i
  th_g    = W2b @ leakyrelu(W2a @ theta, 0.1)      (scalar)
  attn    = softmax(sim2, -1) * (sim2 > th_g)

Sharding: data-parallel over groups; 8 cores x 64 groups each.

Device strategy per core:
  - load q,k natural [128, 128] tiles (partition p holds rows 2p, 2p+1)
  - sigma/diag via DVE/GPSIMD scalar_tensor_tensor with accum
  - scale q by sigma_q, k by sigma_k (fused (x*sig_raw)+x )
  - PE transposes -> qT [qT_even; qT_odd], kT_A (same), kT_B (swapped)
  - 4 matmuls (row-tiled pairs, K=64) -> PSUM sim2 [128, 256] per i-parity
  - rowmax (DVE reduce, negate) -> ACT exp(x - m) with sum accum -> p
  - mask+norm: (psum > theta)*recip_s via dual-op tensor_scalar on PSUM
  - attn = p * mask_scaled ; DMA out with row-interleave unpermute
"""

import sys

for _p in ("/opt/trn_rl_repo", "/opt/pypackages"):
    if _p not in sys.path:
        sys.path.append(_p)

import numpy as np

import concourse.bass as bass
import concourse.mybir as mybir
from concourse.tile import TileContext
from concourse.bass_utils import run_bass_kernel_spmd

F32 = mybir.dt.float32
BF16 = mybir.dt.bfloat16

N_CORES = 8
G = 64          # groups per core
L = 256
D = 64
P = 128

_CACHE = {}


def _build_program(mul_pool=True, sig_pool=False, cp_pool=True, BLK=16, sk_pool=False):
    nc = bass.Bass()

    # ---- I/O ----
    q_in = nc.declare_dram_parameter("q", [G, P, 2 * D], F32, isOutput=False)
    k_in = nc.declare_dram_parameter("k", [G, P, 2 * D], F32, isOutput=False)
    wq_b = nc.declare_dram_parameter("wq_b", [P, D], F32, isOutput=False)
    wk_b = nc.declare_dram_parameter("wk_b", [P, D], F32, isOutput=False)
    w1c = nc.declare_dram_parameter("w1c", [P, 2], F32, isOutput=False)
    w1re = nc.declare_dram_parameter("w1re", [P, P], F32, isOutput=False)
    w1ro = nc.declare_dram_parameter("w1ro", [P, P], F32, isOutput=False)
    w2aT_e = nc.declare_dram_parameter("w2aT_e", [P, 2 * P], F32, isOutput=False)
    w2aT_o = nc.declare_dram_parameter("w2aT_o", [P, 2 * P], F32, isOutput=False)
    w2bc = nc.declare_dram_parameter("w2bc", [P, 2], F32, isOutput=False)
    ident = nc.declare_dram_parameter("ident", [P, P], F32, isOutput=False)
    ones_row = nc.declare_dram_parameter("ones_row", [1, P], F32, isOutput=False)
    out = nc.declare_dram_parameter("attn", [G, P, 2, L], F32, isOutput=True)

    NBLK = G // BLK

    with TileContext(nc) as tc:
        with (
            tc.tile_pool(name="const", bufs=1) as constp,
            tc.tile_pool(name="persist", bufs=1) as persist,
            tc.tile_pool(name="nat", bufs=6) as natp,
            tc.tile_pool(name="scaled", bufs=6) as scaledp,
            tc.tile_pool(name="scratch", bufs=8) as scrp,
            tc.tile_pool(name="tsb", bufs=3) as tsbp,
            tc.tile_pool(name="soft", bufs=6) as softp,
            tc.tile_pool(name="outp", bufs=6) as outp,
            tc.tile_pool(name="ptr", bufs=3, space="PSUM") as ptr,
            tc.tile_pool(name="psim", bufs=3, space="PSUM") as psim,
            tc.tile_pool(name="psm", bufs=2, space="PSUM") as psm,
        ):
            # ---- constants to SBUF ----
            c_wq = constp.tile([P, D], F32, tag="wq")
            nc.sync.dma_start(out=c_wq, in_=wq_b[:, :])
            c_wk = constp.tile([P, D], F32, tag="wk")
            nc.sync.dma_start(out=c_wk, in_=wk_b[:, :])
            c_w1 = constp.tile([P, 2], F32, tag="w1")
            nc.sync.dma_start(out=c_w1, in_=w1c[:, :])
            c_w1re = constp.tile([P, P], F32, tag="w1re")
            nc.sync.dma_start(out=c_w1re, in_=w1re[:, :])
            c_w1ro = constp.tile([P, P], F32, tag="w1ro")
            nc.sync.dma_start(out=c_w1ro, in_=w1ro[:, :])
            c_w2ae = constp.tile([P, 2 * P], F32, tag="w2ae")
            nc.sync.dma_start(out=c_w2ae, in_=w2aT_e[:, :])
            c_w2ao = constp.tile([P, 2 * P], F32, tag="w2ao")
            nc.sync.dma_start(out=c_w2ao, in_=w2aT_o[:, :])
            c_w2b = constp.tile([P, 2], F32, tag="w2b")
            nc.sync.dma_start(out=c_w2b, in_=w2bc[:, :])
            c_id = constp.tile([P, P], F32, tag="ident")
            nc.sync.dma_start(out=c_id, in_=ident[:, :])
            c_ones = constp.tile([1, P], F32, tag="ones")
            nc.sync.dma_start(out=c_ones, in_=ones_row[:, :])

            # pre-touch consts on DVE so later fused ops need <=1 wait
            warm = scrp.tile([P, 2], F32, tag="warm")
            nc.vector.tensor_copy(warm[:, 0:1], c_wq[:, 0:1])
            nc.vector.tensor_copy(warm[:, 1:2], c_wk[:, 0:1])
            # pre-touch weight consts on PE (chained, one new dep per matmul)
            pdum = psm.tile([P, D], F32, tag="smalls")
            for cst in (c_id, c_w1re, c_w1ro, c_w2ae, c_w2ao, c_w2b):
                nc.tensor.matmul(
                    pdum[0:1, 0:1], cst[:, 0:1], c_id[:, 0:1],
                    start=True, stop=True, skip_group_check=True,
                )
            nc.tensor.matmul(
                pdum[0:1, 0:1], c_ones[:, 0:1], c_ones[:, 0:1],
                start=True, stop=True, skip_group_check=True,
            )

            # ---- persistent accumulators ----
            sq_all = persist.tile([P, 2 * G], F32, tag="sq_all")
            qw1_all = persist.tile([P, 2 * G], F32, tag="qw1_all")
            qk_all = persist.tile([P, 2 * G], F32, tag="qk_all")
            th_bc = persist.tile([P, G], F32, tag="th_bc")
            qT_all = persist.tile([P, G * P], F32, tag="qT_all")
            kTA_all = persist.tile([P, G * P], F32, tag="kTA_all")
            kTB_all = persist.tile([P, G * P], F32, tag="kTB_all")

            eng_sig = nc.gpsimd if sig_pool else nc.vector
            eng_cp = nc.gpsimd if cp_pool else nc.vector
            eng_mul = nc.gpsimd if mul_pool else nc.vector

            def phase1_pair(gp):
                q_nat2 = natp.tile([P, 2 * 2 * D], F32, tag="q_nat")
                nc.sync.dma_start(
                    out=q_nat2.rearrange("p (g f) -> p g f", g=2),
                    in_=q_in[2 * gp : 2 * gp + 2].rearrange("g p f -> p g f"),
                )
                k_nat2 = natp.tile([P, 2 * 2 * D], F32, tag="k_nat")
                nc.sync.dma_start(
                    out=k_nat2.rearrange("p (g f) -> p g f", g=2),
                    in_=k_in[2 * gp : 2 * gp + 2].rearrange("g p f -> p g f"),
                )
                for gg in range(2):
                    g = 2 * gp + gg
                    q_nat = q_nat2[:, gg * 2 * D : (gg + 1) * 2 * D]
                    k_nat = k_nat2[:, gg * 2 * D : (gg + 1) * 2 * D]

                    sk_col = scrp.tile([P, 2], F32, tag="sk_col")
                    for c in range(2):
                        sl = slice(c * D, (c + 1) * D)
                        scr = scrp.tile([P, D], F32, tag="sig_scr")
                        eng_sig.scalar_tensor_tensor(
                            out=scr, in0=q_nat[:, sl], scalar=1.0, in1=c_wq,
                            op0=mybir.AluOpType.mult, op1=mybir.AluOpType.mult,
                            accum_out=sq_all[:, 2 * g + c : 2 * g + c + 1],
                        )
                        scr2 = scrp.tile([P, D], F32, tag="sig_scr2")
                        (nc.gpsimd if sk_pool else nc.vector).scalar_tensor_tensor(
                            out=scr2, in0=k_nat[:, sl], scalar=1.0, in1=c_wk,
                            op0=mybir.AluOpType.mult, op1=mybir.AluOpType.mult,
                            accum_out=sk_col[:, c : c + 1],
                        )
                        scr3 = scrp.tile([P, D], F32, tag="dia_scr")
                        eng_sig.scalar_tensor_tensor(
                            out=scr3, in0=q_nat[:, sl], scalar=1.0, in1=k_nat[:, sl],
                            op0=mybir.AluOpType.mult, op1=mybir.AluOpType.mult,
                            accum_out=qk_all[:, 2 * g + c : 2 * g + c + 1],
                        )

                    # w1k broadcast [128, 64] psum
                    w1k_ps = psm.tile([P, D], F32, tag="smalls")
                    nc.tensor.matmul(
                        w1k_ps[0:1, 0:1], k_nat[:, 0:1], k_nat[:, 0:1],
                        start=True, stop=True, skip_group_check=True,
                    )
                    for c in range(2):
                        sl = slice(c * D, (c + 1) * D)
                        w1r = c_w1re if c == 0 else c_w1ro
                        nc.tensor.matmul(
                            w1k_ps, w1r, k_nat[:, sl],
                            start=(c == 0), stop=(c == 1),
                        )
                    w1k_sb = scrp.tile([P, D], F32, tag="w1k_sb")
                    nc.scalar.copy(w1k_sb, w1k_ps)
                    for c in range(2):
                        sl = slice(c * D, (c + 1) * D)
                        scr4 = scrp.tile([P, D], F32, tag="qw1_scr")
                        nc.vector.scalar_tensor_tensor(
                            out=scr4, in0=q_nat[:, sl], scalar=1.0, in1=w1k_sb,
                            op0=mybir.AluOpType.mult, op1=mybir.AluOpType.mult,
                            accum_out=qw1_all[:, 2 * g + c : 2 * g + c + 1],
                        )

                    # scales + swapped copy
                    qs_nat = scaledp.tile([P, 2 * D], F32, tag="qs_nat")
                    ks_nat = scaledp.tile([P, 2 * D], F32, tag="ks_nat")
                    ks_swap = scaledp.tile([P, 2 * D], F32, tag="ks_swap")
                    for c in range(2):
                        sl = slice(c * D, (c + 1) * D)
                        nc.vector.scalar_tensor_tensor(
                            out=qs_nat[:, sl], in0=q_nat[:, sl],
                            scalar=sq_all[:, 2 * g + c : 2 * g + c + 1],
                            in1=q_nat[:, sl],
                            op0=mybir.AluOpType.mult, op1=mybir.AluOpType.add,
                        )
                        nc.vector.scalar_tensor_tensor(
                            out=ks_nat[:, sl], in0=k_nat[:, sl],
                            scalar=sk_col[:, c : c + 1], in1=k_nat[:, sl],
                            op0=mybir.AluOpType.mult, op1=mybir.AluOpType.add,
                        )
                        swp = slice((1 - c) * D, (2 - c) * D)
                        eng_cp.tensor_copy(ks_swap[:, swp], ks_nat[:, sl])

                    # transposes (full [128,128], PSUM base 0)
                    qT_ps = ptr.tile([P, P], F32, tag="tr")
                    nc.tensor.transpose(qT_ps, qs_nat, c_id)
                    nc.scalar.copy(qT_all[:, g * P : (g + 1) * P], qT_ps)

                    kTA_ps = ptr.tile([P, P], F32, tag="tr")
                    nc.tensor.transpose(kTA_ps, ks_nat, c_id)
                    nc.scalar.copy(kTA_all[:, g * P : (g + 1) * P], kTA_ps)

                    kTB_ps = ptr.tile([P, P], F32, tag="tr")
                    nc.tensor.transpose(kTB_ps, ks_swap, c_id)
                    nc.vector.tensor_copy(kTB_all[:, g * P : (g + 1) * P], kTB_ps)

            def mlp_block(blk):
                g0 = blk * BLK
                csl = slice(2 * g0, 2 * (g0 + BLK))
                # theta = qw1 - qk*W1  on this block's columns
                th_blk = tsbp.tile([P, 2 * BLK], F32, tag="th_blk")
                for c in range(2):
                    nc.vector.tensor_scalar(
                        out=th_blk.rearrange("p (g c) -> p c g", c=2)[:, c],
                        in0=qk_all[:, csl].rearrange("p (g c) -> p c g", c=2)[:, c],
                        scalar1=c_w1[:, c : c + 1], scalar2=None,
                        op0=mybir.AluOpType.mult,
                    )
                nc.vector.tensor_tensor(
                    out=th_blk, in0=qw1_all[:, csl], in1=th_blk,
                    op=mybir.AluOpType.subtract,
                )
                # hdn = lrelu(W2a @ theta)
                hdn_sb = tsbp.tile([P, 2 * BLK], F32, tag="hdn")
                for ob in range(2):
                    h_ps = psm.tile([P, BLK], F32, tag="smalls")
                    for c in range(2):
                        w2a = c_w2ae if c == 0 else c_w2ao
                        nc.tensor.matmul(
                            h_ps,
                            w2a[:, ob * P : (ob + 1) * P],
                            th_blk.rearrange("p (g c) -> p c g", c=2)[:, c],
                            start=(c == 0), stop=(c == 1),
                        )
                    rpos = tsbp.tile([P, BLK], F32, tag=f"rpos{ob}")
                    nc.scalar.activation(
                        rpos, h_ps, mybir.ActivationFunctionType.Relu,
                    )
                    h01 = tsbp.tile([P, BLK], F32, tag=f"h01{ob}")
                    nc.vector.tensor_scalar_mul(h01, h_ps, 0.1)
                    nc.vector.scalar_tensor_tensor(
                        out=hdn_sb[:, ob * BLK : (ob + 1) * BLK],
                        in0=rpos, scalar=0.9, in1=h01,
                        op0=mybir.AluOpType.mult, op1=mybir.AluOpType.add,
                    )
                th_ps = psm.tile([BLK, 1], F32, tag="smalls")
                for ob in range(2):
                    nc.tensor.matmul(
                        th_ps,
                        hdn_sb[:, ob * BLK : (ob + 1) * BLK],
                        c_w2b[:, ob : ob + 1],
                        start=(ob == 0), stop=(ob == 1),
                    )
                th_sb = tsbp.tile([BLK, 1], F32, tag="th_col")
                nc.vector.tensor_copy(th_sb, th_ps)
                thr_ps = psm.tile([1, BLK], F32, tag="smalls")
                nc.tensor.transpose(thr_ps, th_sb, c_id[0:BLK, 0:BLK])
                thr_sb = tsbp.tile([1, BLK], F32, tag="th_row")
                nc.vector.tensor_copy(thr_sb, thr_ps)
                thb_ps = psm.tile([P, BLK], F32, tag="smalls")
                nc.tensor.matmul(thb_ps, c_ones, thr_sb, start=True, stop=True)
                nc.vector.tensor_copy(th_bc[:, g0 : g0 + BLK], thb_ps)

            def phase3_group(g):
                qT_sb = qT_all[:, g * P : (g + 1) * P]
                kTA_sb = kTA_all[:, g * P : (g + 1) * P]
                kTB_sb = kTB_all[:, g * P : (g + 1) * P]
                a_full = outp.tile([P, 2 * L], F32, tag="a_sb")
                for ci in range(2):
                    sim_ps = psim.tile([P, L], F32, tag="sim")
                    lhs = qT_sb[ci * D : (ci + 1) * D, :]
                    rA = kTA_sb if ci == 0 else kTB_sb
                    rB = kTB_sb if ci == 0 else kTA_sb
                    nc.tensor.matmul(
                        sim_ps[:, 0:P], lhs, rA[ci * D : (ci + 1) * D, :],
                        start=True, stop=True, tile_position=(ci * D, 0),
                    )
                    nc.tensor.matmul(
                        sim_ps[:, P : 2 * P], lhs, rB[ci * D : (ci + 1) * D, :],
                        start=True, stop=True, tile_position=(ci * D, 0),
                    )
                    neg_m = softp.tile([P, 1], F32, tag="neg_m")
                    nc.vector.tensor_reduce(
                        neg_m, sim_ps, axis=mybir.AxisListType.X,
                        op=mybir.AluOpType.max, negate=True,
                    )
                    s_col = softp.tile([P, 1], F32, tag="s_col")
                    p_sb = softp.tile([P, L], F32, tag="p_sb")
                    p_wr = p_sb[:, :].rearrange("p (j c) -> p c j", c=2)
                    sim_rd = sim_ps[:, :].rearrange("p (c j) -> p c j", c=2)
                    nc.scalar.activation(
                        p_wr, sim_rd, mybir.ActivationFunctionType.Exp,
                        bias=neg_m, scale=1.0, accum_out=s_col,
                    )
                    # t = exp(theta - m): mask threshold in exp-space
                    t_col = softp.tile([P, 1], F32, tag="t_col")
                    nc.scalar.activation(
                        t_col, th_bc[:, g : g + 1],
                        mybir.ActivationFunctionType.Exp, bias=neg_m,
                    )
                    r_col = softp.tile([P, 1], F32, tag="r_col")
                    nc.vector.reciprocal(r_col, s_col)
                    msk = softp.tile([P, L], F32, tag="msk")
                    nc.vector.tensor_scalar(
                        out=msk, in0=p_sb,
                        scalar1=t_col, scalar2=r_col,
                        op0=mybir.AluOpType.is_gt, op1=mybir.AluOpType.mult,
                    )
                    # attn = p * msk (p already true-j ordered)
                    eng_mul.tensor_mul(
                        a_full[:, ci * L : (ci + 1) * L], p_sb, msk
                    )
                nc.sync.dma_start(
                    out=out[g].rearrange("p a b -> p (a b)"), in_=a_full
                )

            # ---- interleaved schedule ----
            for blk in range(NBLK):
                for gp in range(blk * BLK // 2, (blk + 1) * BLK // 2):
                    phase1_pair(gp)
                mlp_block(blk)
                if blk >= 1:
                    for g in range((blk - 1) * BLK, blk * BLK):
                        phase3_group(g)
            for g in range((NBLK - 1) * BLK, G):
                phase3_group(g)

    _split_multi_waits(nc)
    return nc


def _split_multi_waits(nc):
    """Compute-engine instruction structs encode a single sync-wait; move
    extra waits onto standalone NoOps in front of the instruction."""
    import dataclasses

    nid = [0]
    for f in nc.m.functions:
        for bb in f.blocks:
            new = []
            for inst in bb.instructions:
                si = inst.sync_info
                if si is not None and len(si.on_wait) > 1:
                    for w in si.on_wait[:-1]:
                        nid[0] += 1
                        nop = mybir.InstNoOp(
                            name=f"IWS-{nid[0]}",
                            engine=inst.engine,
                            ins=[],
                            outs=[],
                            sync_info=mybir.SyncInfo(on_wait=[w], on_update=[]),
                            bass_nofuse=True,
                        )
                        new.append(nop)
                    inst.sync_info = dataclasses.replace(
                        si, on_wait=[si.on_wait[-1]]
                    )
                new.append(inst)
            bb.instructions[:] = new


def _prep_consts(Wq, bq, Wk, bk, W1, W2a, W2b):
    wq_b = np.broadcast_to(Wq[0][None, :], (P, D)).astype(np.float32).copy()
    wk_b = np.broadcast_to(Wk[0][None, :], (P, D)).astype(np.float32).copy()
    w1c = np.stack([W1[0, 0::2], W1[0, 1::2]], axis=1).astype(np.float32).copy()
    w1re = np.broadcast_to(W1[0, 0::2][:, None], (P, P)).astype(np.float32).copy()
    w1ro = np.broadcast_to(W1[0, 1::2][:, None], (P, P)).astype(np.float32).copy()
    w2aT = np.ascontiguousarray(W2a.T)  # [i, o]
    w2aT_e = np.ascontiguousarray(w2aT[0::2, :])
    w2aT_o = np.ascontiguousarray(w2aT[1::2, :])
    w2bc = np.stack([W2b[0, 0:P], W2b[0, P : 2 * P]], axis=1).astype(np.float32).copy()
    ident = np.eye(P, dtype=np.float32)
    ones_row = np.ones((1, P), dtype=np.float32)
    return dict(
        wq_b=wq_b, wk_b=wk_b, w1c=w1c, w1re=w1re, w1ro=w1ro,
        w2aT_e=w2aT_e, w2aT_o=w2aT_o,
        w2bc=w2bc, ident=ident, ones_row=ones_row,
    )


def kernel(q, k, Wq, bq, Wk, bk, W1, W2a, W2b, _trace=False, _trace_kwargs=None):
    q = np.ascontiguousarray(np.asarray(q, dtype=np.float32))
    k = np.ascontiguousarray(np.asarray(k, dtype=np.float32))
    b, h, B, Lq, Dq = q.shape
    GT = b * h * B
    qf = q.reshape(GT, Lq, Dq)
    kf = k.reshape(GT, Lq, Dq)

    consts = _prep_consts(
        np.asarray(Wq), np.asarray(bq), np.asarray(Wk), np.asarray(bk),
        np.asarray(W1), np.asarray(W2a), np.asarray(W2b),
    )

    if "nc" not in _CACHE:
        _CACHE["nc"] = _build_program()
    nc = _CACHE["nc"]

    gpc = GT // N_CORES
    in_maps = []
    for c in range(N_CORES):
        qs = qf[c * gpc : (c + 1) * gpc].reshape(gpc, P, 2 * Dq)
        ks = kf[c * gpc : (c + 1) * gpc].reshape(gpc, P, 2 * Dq)
        m = {"q": np.ascontiguousarray(qs), "k": np.ascontiguousarray(ks)}
        m.update(consts)
        in_maps.append(m)

    kwargs = {}
    if _trace:
        kwargs["trace"] = True
        if _trace_kwargs:
            kwargs.update(_trace_kwargs)
    res = run_bass_kernel_spmd(nc, in_maps, core_ids=list(range(N_CORES)), **kwargs)
    outs = [r["attn"].reshape(gpc, Lq, Lq) for r in res.results]
    full = np.concatenate(outs, axis=0).reshape(b, h, B, Lq, Lq)
    if _trace:
        _CACHE["last_result"] = res
    return full



# revision 38
# speedup vs baseline: 1.1441x; 1.1441x over previous
"""Trainium2 Bass kernel for nn_Attention_49177375539262 (sparse_attention).

Math (per group g of b*h*B = 512 groups, L=256, D=64):
  sigma_q = q @ Wq^T + 1        [L]
  sigma_k = k @ Wk^T + 1        [L]
  sim     = q @ k^T             [L, L]
  sim2    = sim * outer(sigma_q, sigma_k)
  theta_i = (sim * (1-I)) @ W1 = q @ (k^T @ W1) - (q_i . k_i) * W1_i
  th_g    = W2b @ leakyrelu(W2a @ theta, 0.1)      (scalar)
  attn    = softmax(sim2, -1) * (sim2 > th_g)

Sharding: data-parallel over groups; 8 cores x 64 groups each.

Device strategy per core:
  - load q,k natural [128, 128] tiles (partition p holds rows 2p, 2p+1)
  - sigma/diag via DVE/GPSIMD scalar_tensor_tensor with accum
  - scale q by sigma_q, k by sigma_k (fused (x*sig_raw)+x )
  - PE transposes -> qT [qT_even; qT_odd], kT_A (same), kT_B (swapped)
  - 4 matmuls (row-tiled pairs, K=64) -> PSUM sim2 [128, 256] per i-parity
  - rowmax (DVE reduce, negate) -> ACT exp(x - m) with sum accum -> p
  - mask+norm: (psum > theta)*recip_s via dual-op tensor_scalar on PSUM
  - attn = p * mask_scaled ; DMA out with row-interleave unpermute
"""

import sys

for _p in ("/opt/trn_rl_repo", "/opt/pypackages"):
    if _p not in sys.path:
        sys.path.append(_p)

import numpy as np

import concourse.bass as bass
import concourse.mybir as mybir
from concourse.tile import TileContext
from concourse.bass_utils import run_bass_kernel_spmd

F32 = mybir.dt.float32
BF16 = mybir.dt.bfloat16

N_CORES = 8
G = 64          # groups per core
L = 256
D = 64
P = 128

_CACHE = {}


def _build_program(mul_pool=True, sig_pool=False, cp_pool=True, BLK=16, sk_pool=False):
    nc = bass.Bass()

    # ---- I/O ----
    q_in = nc.declare_dram_parameter("q", [G, P, 2 * D], F32, isOutput=False)
    k_in = nc.declare_dram_parameter("k", [G, P, 2 * D], F32, isOutput=False)
    wq_b = nc.declare_dram_parameter("wq_b", [P, D], F32, isOutput=False)
    wk_b = nc.declare_dram_parameter("wk_b", [P, D], F32, isOutput=False)
    w1c = nc.declare_dram_parameter("w1c", [P, 2], F32, isOutput=False)
    w1re = nc.declare_dram_parameter("w1re", [P, P], F32, isOutput=False)
    w1ro = nc.declare_dram_parameter("w1ro", [P, P], F32, isOutput=False)
    w2aT_e = nc.declare_dram_parameter("w2aT_e", [P, 2 * P], F32, isOutput=False)
    w2aT_o = nc.declare_dram_parameter("w2aT_o", [P, 2 * P], F32, isOutput=False)
    w2bc = nc.declare_dram_parameter("w2bc", [P, 2], F32, isOutput=False)
    ident = nc.declare_dram_parameter("ident", [P, P], F32, isOutput=False)
    ones_row = nc.declare_dram_parameter("ones_row", [1, P], F32, isOutput=False)
    out = nc.declare_dram_parameter("attn", [G, P, 2, L], F32, isOutput=True)

    NBLK = G // BLK

    with TileContext(nc) as tc:
        with (
            tc.tile_pool(name="const", bufs=1) as constp,
            tc.tile_pool(name="persist", bufs=1) as persist,
            tc.tile_pool(name="nat", bufs=6) as natp,
            tc.tile_pool(name="scaled", bufs=6) as scaledp,
            tc.tile_pool(name="scratch", bufs=8) as scrp,
            tc.tile_pool(name="tsb", bufs=3) as tsbp,
            tc.tile_pool(name="soft", bufs=6) as softp,
            tc.tile_pool(name="outp", bufs=6) as outp,
            tc.tile_pool(name="ptr", bufs=3, space="PSUM") as ptr,
            tc.tile_pool(name="psim", bufs=3, space="PSUM") as psim,
            tc.tile_pool(name="psm", bufs=2, space="PSUM") as psm,
        ):
            # ---- constants to SBUF ----
            c_wq = constp.tile([P, D], F32, tag="wq")
            nc.sync.dma_start(out=c_wq, in_=wq_b[:, :])
            c_wk = constp.tile([P, D], F32, tag="wk")
            nc.sync.dma_start(out=c_wk, in_=wk_b[:, :])
            c_w1 = constp.tile([P, 2], F32, tag="w1")
            nc.sync.dma_start(out=c_w1, in_=w1c[:, :])
            c_w1re = constp.tile([P, P], F32, tag="w1re")
            nc.sync.dma_start(out=c_w1re, in_=w1re[:, :])
            c_w1ro = constp.tile([P, P], F32, tag="w1ro")
            nc.sync.dma_start(out=c_w1ro, in_=w1ro[:, :])
            c_w2ae = constp.tile([P, 2 * P], F32, tag="w2ae")
            nc.sync.dma_start(out=c_w2ae, in_=w2aT_e[:, :])
            c_w2ao = constp.tile([P, 2 * P], F32, tag="w2ao")
            nc.sync.dma_start(out=c_w2ao, in_=w2aT_o[:, :])
            c_w2b = constp.tile([P, 2], F32, tag="w2b")
            nc.sync.dma_start(out=c_w2b, in_=w2bc[:, :])
            c_id = constp.tile([P, P], F32, tag="ident")
            nc.sync.dma_start(out=c_id, in_=ident[:, :])
            c_ones = constp.tile([1, P], F32, tag="ones")
            nc.sync.dma_start(out=c_ones, in_=ones_row[:, :])

            # pre-touch consts on DVE so later fused ops need <=1 wait
            warm = scrp.tile([P, 2], F32, tag="warm")
            nc.vector.tensor_copy(warm[:, 0:1], c_wq[:, 0:1])
            nc.vector.tensor_copy(warm[:, 1:2], c_wk[:, 0:1])
            # pre-touch weight consts on PE (chained, one new dep per matmul)
            pdum = psm.tile([P, D], F32, tag="smalls")
            for cst in (c_id, c_w1re, c_w1ro, c_w2ae, c_w2ao, c_w2b):
                nc.tensor.matmul(
                    pdum[0:1, 0:1], cst[:, 0:1], c_id[:, 0:1],
                    start=True, stop=True, skip_group_check=True,
                )
            nc.tensor.matmul(
                pdum[0:1, 0:1], c_ones[:, 0:1], c_ones[:, 0:1],
                start=True, stop=True, skip_group_check=True,
            )

            # ---- persistent accumulators ----
            sq_all = persist.tile([P, 2 * G], F32, tag="sq_all")
            qw1_all = persist.tile([P, 2 * G], F32, tag="qw1_all")
            qk_all = persist.tile([P, 2 * G], F32, tag="qk_all")
            th_bc = persist.tile([P, G], F32, tag="th_bc")
            qT_all = persist.tile([P, G * P], F32, tag="qT_all")
            kTA_all = persist.tile([P, G * P], F32, tag="kTA_all")
            kTB_all = persist.tile([P, G * P], F32, tag="kTB_all")

            eng_sig = nc.gpsimd if sig_pool else nc.vector
            eng_cp = nc.gpsimd if cp_pool else nc.vector
            eng_mul = nc.gpsimd if mul_pool else nc.vector

            def phase1_pair(gp):
                q_nat2 = natp.tile([P, 2 * 2 * D], F32, tag="q_nat")
                nc.sync.dma_start(
                    out=q_nat2.rearrange("p (g f) -> p g f", g=2),
                    in_=q_in[2 * gp : 2 * gp + 2].rearrange("g p f -> p g f"),
                )
                k_nat2 = natp.tile([P, 2 * 2 * D], F32, tag="k_nat")
                nc.sync.dma_start(
                    out=k_nat2.rearrange("p (g f) -> p g f", g=2),
                    in_=k_in[2 * gp : 2 * gp + 2].rearrange("g p f -> p g f"),
                )
                for gg in range(2):
                    g = 2 * gp + gg
                    q_nat = q_nat2[:, gg * 2 * D : (gg + 1) * 2 * D]
                    k_nat = k_nat2[:, gg * 2 * D : (gg + 1) * 2 * D]

                    sk_col = scrp.tile([P, 2], F32, tag="sk_col")
                    for c in range(2):
                        sl = slice(c * D, (c + 1) * D)
                        scr = scrp.tile([P, D], F32, tag="sig_scr")
                        eng_sig.scalar_tensor_tensor(
                            out=scr, in0=q_nat[:, sl], scalar=1.0, in1=c_wq,
                            op0=mybir.AluOpType.mult, op1=mybir.AluOpType.mult,
                            accum_out=sq_all[:, 2 * g + c : 2 * g + c + 1],
                        )
                        scr2 = scrp.tile([P, D], F32, tag="sig_scr2")
                        (nc.gpsimd if sk_pool else nc.vector).scalar_tensor_tensor(
                            out=scr2, in0=k_nat[:, sl], scalar=1.0, in1=c_wk,
                            op0=mybir.AluOpType.mult, op1=mybir.AluOpType.mult,
                            accum_out=sk_col[:, c : c + 1],
                        )
                        scr3 = scrp.tile([P, D], F32, tag="dia_scr")
                        eng_sig.scalar_tensor_tensor(
                            out=scr3, in0=q_nat[:, sl], scalar=1.0, in1=k_nat[:, sl],
                            op0=mybir.AluOpType.mult, op1=mybir.AluOpType.mult,
                            accum_out=qk_all[:, 2 * g + c : 2 * g + c + 1],
                        )

                    # w1k broadcast [128, 64] psum
                    w1k_ps = psm.tile([P, D], F32, tag="smalls")
                    nc.tensor.matmul(
                        w1k_ps[0:1, 0:1], k_nat[:, 0:1], k_nat[:, 0:1],
                        start=True, stop=True, skip_group_check=True,
                    )
                    for c in range(2):
                        sl = slice(c * D, (c + 1) * D)
                        w1r = c_w1re if c == 0 else c_w1ro
                        nc.tensor.matmul(
                            w1k_ps, w1r, k_nat[:, sl],
                            start=(c == 0), stop=(c == 1),
                        )
                    w1k_sb = scrp.tile([P, D], F32, tag="w1k_sb")
                    nc.scalar.copy(w1k_sb, w1k_ps)
                    for c in range(2):
                        sl = slice(c * D, (c + 1) * D)
                        scr4 = scrp.tile([P, D], F32, tag="qw1_scr")
                        nc.vector.scalar_tensor_tensor(
                            out=scr4, in0=q_nat[:, sl], scalar=1.0, in1=w1k_sb,
                            op0=mybir.AluOpType.mult, op1=mybir.AluOpType.mult,
                            accum_out=qw1_all[:, 2 * g + c : 2 * g + c + 1],
                        )

                    # scales as tensor_scalar (Pool-legal) with 1+sigma'
                    sq1 = scrp.tile([P, 2], F32, tag="sq1")
                    nc.gpsimd.tensor_scalar(
                        out=sq1, in0=sq_all[:, 2 * g : 2 * g + 2],
                        scalar1=1.0, scalar2=None, op0=mybir.AluOpType.add,
                    )
                    sk1 = scrp.tile([P, 2], F32, tag="sk1")
                    nc.gpsimd.tensor_scalar(
                        out=sk1, in0=sk_col,
                        scalar1=1.0, scalar2=None, op0=mybir.AluOpType.add,
                    )
                    qs_nat = scaledp.tile([P, 2 * D], F32, tag="qs_nat")
                    ks_nat = scaledp.tile([P, 2 * D], F32, tag="ks_nat")
                    ks_swap = scaledp.tile([P, 2 * D], F32, tag="ks_swap")
                    for c in range(2):
                        sl = slice(c * D, (c + 1) * D)
                        nc.gpsimd.tensor_scalar(
                            out=qs_nat[:, sl], in0=q_nat[:, sl],
                            scalar1=sq1[:, c : c + 1], scalar2=None,
                            op0=mybir.AluOpType.mult,
                        )
                        nc.gpsimd.tensor_scalar(
                            out=ks_nat[:, sl], in0=k_nat[:, sl],
                            scalar1=sk1[:, c : c + 1], scalar2=None,
                            op0=mybir.AluOpType.mult,
                        )
                        swp = slice((1 - c) * D, (2 - c) * D)
                        eng_cp.tensor_copy(ks_swap[:, swp], ks_nat[:, sl])

                    # transposes (full [128,128], PSUM base 0)
                    qT_ps = ptr.tile([P, P], F32, tag="tr")
                    nc.tensor.transpose(qT_ps, qs_nat, c_id)
                    nc.scalar.copy(qT_all[:, g * P : (g + 1) * P], qT_ps)

                    kTA_ps = ptr.tile([P, P], F32, tag="tr")
                    nc.tensor.transpose(kTA_ps, ks_nat, c_id)
                    nc.scalar.copy(kTA_all[:, g * P : (g + 1) * P], kTA_ps)

                    kTB_ps = ptr.tile([P, P], F32, tag="tr")
                    nc.tensor.transpose(kTB_ps, ks_swap, c_id)
                    nc.scalar.copy(kTB_all[:, g * P : (g + 1) * P], kTB_ps)

            def mlp_block(blk):
                g0 = blk * BLK
                csl = slice(2 * g0, 2 * (g0 + BLK))
                # theta = qw1 - qk*W1  on this block's columns
                th_blk = tsbp.tile([P, 2 * BLK], F32, tag="th_blk")
                for c in range(2):
                    nc.vector.tensor_scalar(
                        out=th_blk.rearrange("p (g c) -> p c g", c=2)[:, c],
                        in0=qk_all[:, csl].rearrange("p (g c) -> p c g", c=2)[:, c],
                        scalar1=c_w1[:, c : c + 1], scalar2=None,
                        op0=mybir.AluOpType.mult,
                    )
                nc.vector.tensor_tensor(
                    out=th_blk, in0=qw1_all[:, csl], in1=th_blk,
                    op=mybir.AluOpType.subtract,
                )
                # hdn = lrelu(W2a @ theta)
                hdn_sb = tsbp.tile([P, 2 * BLK], F32, tag="hdn")
                for ob in range(2):
                    h_ps = psm.tile([P, BLK], F32, tag="smalls")
                    for c in range(2):
                        w2a = c_w2ae if c == 0 else c_w2ao
                        nc.tensor.matmul(
                            h_ps,
                            w2a[:, ob * P : (ob + 1) * P],
                            th_blk.rearrange("p (g c) -> p c g", c=2)[:, c],
                            start=(c == 0), stop=(c == 1),
                        )
                    rpos = tsbp.tile([P, BLK], F32, tag=f"rpos{ob}")
                    nc.scalar.activation(
                        rpos, h_ps, mybir.ActivationFunctionType.Relu,
                    )
                    h01 = tsbp.tile([P, BLK], F32, tag=f"h01{ob}")
                    nc.vector.tensor_scalar_mul(h01, h_ps, 0.1)
                    nc.vector.scalar_tensor_tensor(
                        out=hdn_sb[:, ob * BLK : (ob + 1) * BLK],
                        in0=rpos, scalar=0.9, in1=h01,
                        op0=mybir.AluOpType.mult, op1=mybir.AluOpType.add,
                    )
                th_ps = psm.tile([BLK, 1], F32, tag="smalls")
                for ob in range(2):
                    nc.tensor.matmul(
                        th_ps,
                        hdn_sb[:, ob * BLK : (ob + 1) * BLK],
                        c_w2b[:, ob : ob + 1],
                        start=(ob == 0), stop=(ob == 1),
                    )
                th_sb = tsbp.tile([BLK, 1], F32, tag="th_col")
                nc.vector.tensor_copy(th_sb, th_ps)
                thr_ps = psm.tile([1, BLK], F32, tag="smalls")
                nc.tensor.transpose(thr_ps, th_sb, c_id[0:BLK, 0:BLK])
                thr_sb = tsbp.tile([1, BLK], F32, tag="th_row")
                nc.vector.tensor_copy(thr_sb, thr_ps)
                thb_ps = psm.tile([P, BLK], F32, tag="smalls")
                nc.tensor.matmul(thb_ps, c_ones, thr_sb, start=True, stop=True)
                nc.vector.tensor_copy(th_bc[:, g0 : g0 + BLK], thb_ps)

            def phase3_group(g):
                qT_sb = qT_all[:, g * P : (g + 1) * P]
                kTA_sb = kTA_all[:, g * P : (g + 1) * P]
                kTB_sb = kTB_all[:, g * P : (g + 1) * P]
                a_full = outp.tile([P, 2 * L], F32, tag="a_sb")
                for ci in range(2):
                    sim_ps = psim.tile([P, L], F32, tag="sim")
                    lhs = qT_sb[ci * D : (ci + 1) * D, :]
                    rA = kTA_sb if ci == 0 else kTB_sb
                    rB = kTB_sb if ci == 0 else kTA_sb
                    nc.tensor.matmul(
                        sim_ps[:, 0:P], lhs, rA[ci * D : (ci + 1) * D, :],
                        start=True, stop=True, tile_position=(ci * D, 0),
                    )
                    nc.tensor.matmul(
                        sim_ps[:, P : 2 * P], lhs, rB[ci * D : (ci + 1) * D, :],
                        start=True, stop=True, tile_position=(ci * D, 0),
                    )
                    neg_m = softp.tile([P, 1], F32, tag="neg_m")
                    nc.vector.tensor_reduce(
                        neg_m, sim_ps, axis=mybir.AxisListType.X,
                        op=mybir.AluOpType.max, negate=True,
                    )
                    s_col = softp.tile([P, 1], F32, tag="s_col")
                    p_sb = softp.tile([P, L], F32, tag="p_sb")
                    p_wr = p_sb[:, :].rearrange("p (j c) -> p c j", c=2)
                    sim_rd = sim_ps[:, :].rearrange("p (c j) -> p c j", c=2)
                    nc.scalar.activation(
                        p_wr, sim_rd, mybir.ActivationFunctionType.Exp,
                        bias=neg_m, scale=1.0, accum_out=s_col,
                    )
                    # t = exp(theta - m): mask threshold in exp-space
                    t_col = softp.tile([P, 1], F32, tag="t_col")
                    nc.scalar.activation(
                        t_col, th_bc[:, g : g + 1],
                        mybir.ActivationFunctionType.Exp, bias=neg_m,
                    )
                    r_col = softp.tile([P, 1], F32, tag="r_col")
                    nc.vector.reciprocal(r_col, s_col)
                    msk = softp.tile([P, L], F32, tag="msk")
                    nc.vector.tensor_scalar(
                        out=msk, in0=p_sb,
                        scalar1=t_col, scalar2=r_col,
                        op0=mybir.AluOpType.is_gt, op1=mybir.AluOpType.mult,
                    )
                    # attn = p * msk (p already true-j ordered)
                    eng_mul.tensor_mul(
                        a_full[:, ci * L : (ci + 1) * L], p_sb, msk
                    )
                nc.sync.dma_start(
                    out=out[g].rearrange("p a b -> p (a b)"), in_=a_full
                )

            # ---- interleaved schedule ----
            for blk in range(NBLK):
                for gp in range(blk * BLK // 2, (blk + 1) * BLK // 2):
                    phase1_pair(gp)
                mlp_block(blk)
                if blk >= 1:
                    for g in range((blk - 1) * BLK, blk * BLK):
                        phase3_group(g)
            for g in range((NBLK - 1) * BLK, G):
                phase3_group(g)

    _split_multi_waits(nc)
    return nc


def _split_multi_waits(nc):
    """Compute-engine instruction structs encode a single sync-wait; move
    extra waits onto standalone NoOps in front of the instruction."""
    import dataclasses

    nid = [0]
    for f in nc.m.functions:
        for bb in f.blocks:
            new = []
            for inst in bb.instructions:
                si = inst.sync_info
                if si is not None and len(si.on_wait) > 1:
                    for w in si.on_wait[:-1]:
                        nid[0] += 1
                        nop = mybir.InstNoOp(
                            name=f"IWS-{nid[0]}",
                            engine=inst.engine,
                            ins=[],
                            outs=[],
                            sync_info=mybir.SyncInfo(on_wait=[w], on_update=[]),
                            bass_nofuse=True,
                        )
                        new.append(nop)
                    inst.sync_info = dataclasses.replace(
                        si, on_wait=[si.on_wait[-1]]
                    )
                new.append(inst)
            bb.instructions[:] = new


def _prep_consts(Wq, bq, Wk, bk, W1, W2a, W2b):
    wq_b = np.broadcast_to(Wq[0][None, :], (P, D)).astype(np.float32).copy()
    wk_b = np.broadcast_to(Wk[0][None, :], (P, D)).astype(np.float32).copy()
    w1c = np.stack([W1[0, 0::2], W1[0, 1::2]], axis=1).astype(np.float32).copy()
    w1re = np.broadcast_to(W1[0, 0::2][:, None], (P, P)).astype(np.float32).copy()
    w1ro = np.broadcast_to(W1[0, 1::2][:, None], (P, P)).astype(np.float32).copy()
    w2aT = np.ascontiguousarray(W2a.T)  # [i, o]
    w2aT_e = np.ascontiguousarray(w2aT[0::2, :])
    w2aT_o = np.ascontiguousarray(w2aT[1::2, :])
    w2bc = np.stack([W2b[0, 0:P], W2b[0, P : 2 * P]], axis=1).astype(np.float32).copy()
    ident = np.eye(P, dtype=np.float32)
    ones_row = np.ones((1, P), dtype=np.float32)
    return dict(
        wq_b=wq_b, wk_b=wk_b, w1c=w1c, w1re=w1re, w1ro=w1ro,
        w2aT_e=w2aT_e, w2aT_o=w2aT_o,
        w2bc=w2bc, ident=ident, ones_row=ones_row,
    )


def kernel(q, k, Wq, bq, Wk, bk, W1, W2a, W2b, _trace=False, _trace_kwargs=None):
    q = np.ascontiguousarray(np.asarray(q, dtype=np.float32))
    k = np.ascontiguousarray(np.asarray(k, dtype=np.float32))
    b, h, B, Lq, Dq = q.shape
    GT = b * h * B
    qf = q.reshape(GT, Lq, Dq)
    kf = k.reshape(GT, Lq, Dq)

    consts = _prep_consts(
        np.asarray(Wq), np.asarray(bq), np.asarray(Wk), np.asarray(bk),
        np.asarray(W1), np.asarray(W2a), np.asarray(W2b),
    )

    if "nc" not in _CACHE:
        _CACHE["nc"] = _build_program()
    nc = _CACHE["nc"]

    gpc = GT // N_CORES
    in_maps = []
    for c in range(N_CORES):
        qs = qf[c * gpc : (c + 1) * gpc].reshape(gpc, P, 2 * Dq)
        ks = kf[c * gpc : (c + 1) * gpc].reshape(gpc, P, 2 * Dq)
        m = {"q": np.ascontiguousarray(qs), "k": np.ascontiguousarray(ks)}
        m.update(consts)
        in_maps.append(m)

    kwargs = {}
    if _trace:
        kwargs["trace"] = True
        if _trace_kwargs:
            kwargs.update(_trace_kwargs)
    res = run_bass_kernel_spmd(nc, in_maps, core_ids=list(range(N_CORES)), **kwargs)
    outs = [r["attn"].reshape(gpc, Lq, Lq) for r in res.results]
    full = np.concatenate(outs, axis=0).reshape(b, h, B, Lq, Lq)
    if _trace:
        _CACHE["last_result"] = res
    return full



# revision 43
# speedup vs baseline: 1.2288x; 1.0740x over previous
"""Trainium2 Bass kernel for nn_Attention_49177375539262 (sparse_attention).

Math (per group g of b*h*B = 512 groups, L=256, D=64):
  sigma_q = q @ Wq^T + 1        [L]
  sigma_k = k @ Wk^T + 1        [L]
  sim     = q @ k^T             [L, L]
  sim2    = sim * outer(sigma_q, sigma_k)
  theta_i = (sim * (1-I)) @ W1 = q @ (k^T @ W1) - (q_i . k_i) * W1_i
  th_g    = W2b @ leakyrelu(W2a @ theta, 0.1)      (scalar)
  attn    = softmax(sim2, -1) * (sim2 > th_g)

Sharding: data-parallel over groups; 8 cores x 64 groups each.

Device strategy per core:
  - load q,k natural [128, 128] tiles (partition p holds rows 2p, 2p+1)
  - sigma/diag via DVE/GPSIMD scalar_tensor_tensor with accum
  - scale q by sigma_q, k by sigma_k (fused (x*sig_raw)+x )
  - PE transposes -> qT [qT_even; qT_odd], kT_A (same), kT_B (swapped)
  - 4 matmuls (row-tiled pairs, K=64) -> PSUM sim2 [128, 256] per i-parity
  - rowmax (DVE reduce, negate) -> ACT exp(x - m) with sum accum -> p
  - mask+norm: (psum > theta)*recip_s via dual-op tensor_scalar on PSUM
  - attn = p * mask_scaled ; DMA out with row-interleave unpermute
"""

import sys

for _p in ("/opt/trn_rl_repo", "/opt/pypackages"):
    if _p not in sys.path:
        sys.path.append(_p)

import numpy as np

import concourse.bass as bass
import concourse.mybir as mybir
from concourse.tile import TileContext
from concourse.bass_utils import run_bass_kernel_spmd

F32 = mybir.dt.float32
BF16 = mybir.dt.bfloat16

N_CORES = 8
G = 64          # groups per core
L = 256
D = 64
P = 128

_CACHE = {}


def _build_program(mul_pool=True, sig_pool=False, cp_pool=True, BLK=16, sk_pool=False,
                   qs_pool=False, ks_pool=True):
    nc = bass.Bass()

    # ---- I/O ----
    q_in = nc.declare_dram_parameter("q", [G, P, 2 * D], F32, isOutput=False)
    k_in = nc.declare_dram_parameter("k", [G, P, 2 * D], F32, isOutput=False)
    wq_b = nc.declare_dram_parameter("wq_b", [P, D], F32, isOutput=False)
    wk_b = nc.declare_dram_parameter("wk_b", [P, D], F32, isOutput=False)
    w1c = nc.declare_dram_parameter("w1c", [P, 2], F32, isOutput=False)
    w1re = nc.declare_dram_parameter("w1re", [P, P], F32, isOutput=False)
    w1ro = nc.declare_dram_parameter("w1ro", [P, P], F32, isOutput=False)
    w2aT_e = nc.declare_dram_parameter("w2aT_e", [P, 2 * P], F32, isOutput=False)
    w2aT_o = nc.declare_dram_parameter("w2aT_o", [P, 2 * P], F32, isOutput=False)
    w2bc = nc.declare_dram_parameter("w2bc", [P, 2], F32, isOutput=False)
    ident = nc.declare_dram_parameter("ident", [P, P], F32, isOutput=False)
    ones_row = nc.declare_dram_parameter("ones_row", [1, P], F32, isOutput=False)
    out = nc.declare_dram_parameter("attn", [G, P, 2, L], F32, isOutput=True)

    NBLK = G // BLK

    with TileContext(nc) as tc:
        with (
            tc.tile_pool(name="const", bufs=1) as constp,
            tc.tile_pool(name="persist", bufs=1) as persist,
            tc.tile_pool(name="nat", bufs=6) as natp,
            tc.tile_pool(name="scaled", bufs=6) as scaledp,
            tc.tile_pool(name="scratch", bufs=8) as scrp,
            tc.tile_pool(name="tsb", bufs=3) as tsbp,
            tc.tile_pool(name="soft", bufs=6) as softp,
            tc.tile_pool(name="outp", bufs=6) as outp,
            tc.tile_pool(name="ptr", bufs=3, space="PSUM") as ptr,
            tc.tile_pool(name="psim", bufs=3, space="PSUM") as psim,
            tc.tile_pool(name="psm", bufs=2, space="PSUM") as psm,
        ):
            # ---- constants to SBUF ----
            c_wq = constp.tile([P, D], F32, tag="wq")
            nc.sync.dma_start(out=c_wq, in_=wq_b[:, :])
            c_wk = constp.tile([P, D], F32, tag="wk")
            nc.sync.dma_start(out=c_wk, in_=wk_b[:, :])
            c_w1 = constp.tile([P, 2], F32, tag="w1")
            nc.sync.dma_start(out=c_w1, in_=w1c[:, :])
            c_w1re = constp.tile([P, P], F32, tag="w1re")
            nc.sync.dma_start(out=c_w1re, in_=w1re[:, :])
            c_w1ro = constp.tile([P, P], F32, tag="w1ro")
            nc.sync.dma_start(out=c_w1ro, in_=w1ro[:, :])
            c_w2ae = constp.tile([P, 2 * P], F32, tag="w2ae")
            nc.sync.dma_start(out=c_w2ae, in_=w2aT_e[:, :])
            c_w2ao = constp.tile([P, 2 * P], F32, tag="w2ao")
            nc.sync.dma_start(out=c_w2ao, in_=w2aT_o[:, :])
            c_w2b = constp.tile([P, 2], F32, tag="w2b")
            nc.sync.dma_start(out=c_w2b, in_=w2bc[:, :])
            c_id = constp.tile([P, P], F32, tag="ident")
            nc.sync.dma_start(out=c_id, in_=ident[:, :])
            c_ones = constp.tile([1, P], F32, tag="ones")
            nc.sync.dma_start(out=c_ones, in_=ones_row[:, :])

            # pre-touch consts on DVE so later fused ops need <=1 wait
            warm = scrp.tile([P, 2], F32, tag="warm")
            nc.vector.tensor_copy(warm[:, 0:1], c_wq[:, 0:1])
            nc.vector.tensor_copy(warm[:, 1:2], c_wk[:, 0:1])
            # pre-touch weight consts on PE (chained, one new dep per matmul)
            pdum = psm.tile([P, D], F32, tag="smalls")
            for cst in (c_id, c_w1re, c_w1ro, c_w2ae, c_w2ao, c_w2b):
                nc.tensor.matmul(
                    pdum[0:1, 0:1], cst[:, 0:1], c_id[:, 0:1],
                    start=True, stop=True, skip_group_check=True,
                )
            nc.tensor.matmul(
                pdum[0:1, 0:1], c_ones[:, 0:1], c_ones[:, 0:1],
                start=True, stop=True, skip_group_check=True,
            )

            # ---- persistent accumulators ----
            sq_all = persist.tile([P, 2 * G], F32, tag="sq_all")
            qw1_all = persist.tile([P, 2 * G], F32, tag="qw1_all")
            qk_all = persist.tile([P, 2 * G], F32, tag="qk_all")
            th_bc = persist.tile([P, G], F32, tag="th_bc")
            qT_all = persist.tile([P, G * P], F32, tag="qT_all")
            kTA_all = persist.tile([P, G * P], F32, tag="kTA_all")
            kTB_all = persist.tile([P, G * P], F32, tag="kTB_all")

            eng_sig = nc.gpsimd if sig_pool else nc.vector
            eng_qs = nc.gpsimd if qs_pool else nc.vector
            eng_ks = nc.gpsimd if ks_pool else nc.vector
            eng_cp = nc.gpsimd if cp_pool else nc.vector
            eng_mul = nc.gpsimd if mul_pool else nc.vector

            def phase1_pair(gp):
                q_nat2 = natp.tile([P, 2 * 2 * D], F32, tag="q_nat")
                nc.sync.dma_start(
                    out=q_nat2.rearrange("p (g f) -> p g f", g=2),
                    in_=q_in[2 * gp : 2 * gp + 2].rearrange("g p f -> p g f"),
                )
                k_nat2 = natp.tile([P, 2 * 2 * D], F32, tag="k_nat")
                nc.sync.dma_start(
                    out=k_nat2.rearrange("p (g f) -> p g f", g=2),
                    in_=k_in[2 * gp : 2 * gp + 2].rearrange("g p f -> p g f"),
                )
                for gg in range(2):
                    g = 2 * gp + gg
                    q_nat = q_nat2[:, gg * 2 * D : (gg + 1) * 2 * D]
                    k_nat = k_nat2[:, gg * 2 * D : (gg + 1) * 2 * D]

                    sk_col = scrp.tile([P, 2], F32, tag="sk_col")
                    for c in range(2):
                        sl = slice(c * D, (c + 1) * D)
                        scr = scrp.tile([P, D], F32, tag="sig_scr")
                        eng_sig.scalar_tensor_tensor(
                            out=scr, in0=q_nat[:, sl], scalar=1.0, in1=c_wq,
                            op0=mybir.AluOpType.mult, op1=mybir.AluOpType.mult,
                            accum_out=sq_all[:, 2 * g + c : 2 * g + c + 1],
                        )
                        scr2 = scrp.tile([P, D], F32, tag="sig_scr2")
                        (nc.gpsimd if sk_pool else nc.vector).scalar_tensor_tensor(
                            out=scr2, in0=k_nat[:, sl], scalar=1.0, in1=c_wk,
                            op0=mybir.AluOpType.mult, op1=mybir.AluOpType.mult,
                            accum_out=sk_col[:, c : c + 1],
                        )
                        scr3 = scrp.tile([P, D], F32, tag="dia_scr")
                        eng_sig.scalar_tensor_tensor(
                            out=scr3, in0=q_nat[:, sl], scalar=1.0, in1=k_nat[:, sl],
                            op0=mybir.AluOpType.mult, op1=mybir.AluOpType.mult,
                            accum_out=qk_all[:, 2 * g + c : 2 * g + c + 1],
                        )

                    # w1k broadcast [128, 64] psum
                    w1k_ps = psm.tile([P, D], F32, tag="smalls")
                    nc.tensor.matmul(
                        w1k_ps[0:1, 0:1], k_nat[:, 0:1], k_nat[:, 0:1],
                        start=True, stop=True, skip_group_check=True,
                    )
                    for c in range(2):
                        sl = slice(c * D, (c + 1) * D)
                        w1r = c_w1re if c == 0 else c_w1ro
                        nc.tensor.matmul(
                            w1k_ps, w1r, k_nat[:, sl],
                            start=(c == 0), stop=(c == 1),
                        )
                    w1k_sb = scrp.tile([P, D], F32, tag="w1k_sb")
                    nc.scalar.copy(w1k_sb, w1k_ps)
                    for c in range(2):
                        sl = slice(c * D, (c + 1) * D)
                        scr4 = scrp.tile([P, D], F32, tag="qw1_scr")
                        nc.vector.scalar_tensor_tensor(
                            out=scr4, in0=q_nat[:, sl], scalar=1.0, in1=w1k_sb,
                            op0=mybir.AluOpType.mult, op1=mybir.AluOpType.mult,
                            accum_out=qw1_all[:, 2 * g + c : 2 * g + c + 1],
                        )

                    # scales as tensor_scalar (Pool-legal) with 1+sigma'
                    sq1 = scrp.tile([P, 2], F32, tag="sq1")
                    nc.gpsimd.tensor_scalar(
                        out=sq1, in0=sq_all[:, 2 * g : 2 * g + 2],
                        scalar1=1.0, scalar2=None, op0=mybir.AluOpType.add,
                    )
                    sk1 = scrp.tile([P, 2], F32, tag="sk1")
                    nc.gpsimd.tensor_scalar(
                        out=sk1, in0=sk_col,
                        scalar1=1.0, scalar2=None, op0=mybir.AluOpType.add,
                    )
                    qs_nat = scaledp.tile([P, 2 * D], F32, tag="qs_nat")
                    ks_nat = scaledp.tile([P, 2 * D], F32, tag="ks_nat")
                    ks_swap = scaledp.tile([P, 2 * D], F32, tag="ks_swap")
                    for c in range(2):
                        sl = slice(c * D, (c + 1) * D)
                        eng_qs.tensor_scalar(
                            out=qs_nat[:, sl], in0=q_nat[:, sl],
                            scalar1=sq1[:, c : c + 1], scalar2=None,
                            op0=mybir.AluOpType.mult,
                        )
                        eng_ks.tensor_scalar(
                            out=ks_nat[:, sl], in0=k_nat[:, sl],
                            scalar1=sk1[:, c : c + 1], scalar2=None,
                            op0=mybir.AluOpType.mult,
                        )
                        swp = slice((1 - c) * D, (2 - c) * D)
                        eng_cp.tensor_copy(ks_swap[:, swp], ks_nat[:, sl])

                    # transposes (full [128,128], PSUM base 0)
                    qT_ps = ptr.tile([P, P], F32, tag="tr")
                    nc.tensor.transpose(qT_ps, qs_nat, c_id)
                    nc.scalar.copy(qT_all[:, g * P : (g + 1) * P], qT_ps)

                    kTA_ps = ptr.tile([P, P], F32, tag="tr")
                    nc.tensor.transpose(kTA_ps, ks_nat, c_id)
                    nc.scalar.copy(kTA_all[:, g * P : (g + 1) * P], kTA_ps)

                    kTB_ps = ptr.tile([P, P], F32, tag="tr")
                    nc.tensor.transpose(kTB_ps, ks_swap, c_id)
                    nc.scalar.copy(kTB_all[:, g * P : (g + 1) * P], kTB_ps)

            def mlp_block(blk):
                g0 = blk * BLK
                csl = slice(2 * g0, 2 * (g0 + BLK))
                # theta = qw1 - qk*W1  on this block's columns
                th_blk = tsbp.tile([P, 2 * BLK], F32, tag="th_blk")
                for c in range(2):
                    nc.vector.tensor_scalar(
                        out=th_blk.rearrange("p (g c) -> p c g", c=2)[:, c],
                        in0=qk_all[:, csl].rearrange("p (g c) -> p c g", c=2)[:, c],
                        scalar1=c_w1[:, c : c + 1], scalar2=None,
                        op0=mybir.AluOpType.mult,
                    )
                nc.vector.tensor_tensor(
                    out=th_blk, in0=qw1_all[:, csl], in1=th_blk,
                    op=mybir.AluOpType.subtract,
                )
                # hdn = lrelu(W2a @ theta)
                hdn_sb = tsbp.tile([P, 2 * BLK], F32, tag="hdn")
                for ob in range(2):
                    h_ps = psm.tile([P, BLK], F32, tag="smalls")
                    for c in range(2):
                        w2a = c_w2ae if c == 0 else c_w2ao
                        nc.tensor.matmul(
                            h_ps,
                            w2a[:, ob * P : (ob + 1) * P],
                            th_blk.rearrange("p (g c) -> p c g", c=2)[:, c],
                            start=(c == 0), stop=(c == 1),
                        )
                    rpos = tsbp.tile([P, BLK], F32, tag=f"rpos{ob}")
                    nc.scalar.activation(
                        rpos, h_ps, mybir.ActivationFunctionType.Relu,
                    )
                    h01 = tsbp.tile([P, BLK], F32, tag=f"h01{ob}")
                    nc.vector.tensor_scalar_mul(h01, h_ps, 0.1)
                    nc.vector.scalar_tensor_tensor(
                        out=hdn_sb[:, ob * BLK : (ob + 1) * BLK],
                        in0=rpos, scalar=0.9, in1=h01,
                        op0=mybir.AluOpType.mult, op1=mybir.AluOpType.add,
                    )
                th_ps = psm.tile([BLK, 1], F32, tag="smalls")
                for ob in range(2):
                    nc.tensor.matmul(
                        th_ps,
                        hdn_sb[:, ob * BLK : (ob + 1) * BLK],
                        c_w2b[:, ob : ob + 1],
                        start=(ob == 0), stop=(ob == 1),
                    )
                th_sb = tsbp.tile([BLK, 1], F32, tag="th_col")
                nc.vector.tensor_copy(th_sb, th_ps)
                thr_ps = psm.tile([1, BLK], F32, tag="smalls")
                nc.tensor.transpose(thr_ps, th_sb, c_id[0:BLK, 0:BLK])
                thr_sb = tsbp.tile([1, BLK], F32, tag="th_row")
                nc.vector.tensor_copy(thr_sb, thr_ps)
                thb_ps = psm.tile([P, BLK], F32, tag="smalls")
                nc.tensor.matmul(thb_ps, c_ones, thr_sb, start=True, stop=True)
                nc.vector.tensor_copy(th_bc[:, g0 : g0 + BLK], thb_ps)

            def phase3_group(g):
                qT_sb = qT_all[:, g * P : (g + 1) * P]
                kTA_sb = kTA_all[:, g * P : (g + 1) * P]
                kTB_sb = kTB_all[:, g * P : (g + 1) * P]
                a_full = outp.tile([P, 2 * L], F32, tag="a_sb")
                for ci in range(2):
                    sim_ps = psim.tile([P, L], F32, tag="sim")
                    lhs = qT_sb[ci * D : (ci + 1) * D, :]
                    rA = kTA_sb if ci == 0 else kTB_sb
                    rB = kTB_sb if ci == 0 else kTA_sb
                    nc.tensor.matmul(
                        sim_ps[:, 0:P], lhs, rA[ci * D : (ci + 1) * D, :],
                        start=True, stop=True, tile_position=(ci * D, 0),
                    )
                    nc.tensor.matmul(
                        sim_ps[:, P : 2 * P], lhs, rB[ci * D : (ci + 1) * D, :],
                        start=True, stop=True, tile_position=(ci * D, 0),
                    )
                    neg_m = softp.tile([P, 1], F32, tag="neg_m")
                    nc.vector.tensor_reduce(
                        neg_m, sim_ps, axis=mybir.AxisListType.X,
                        op=mybir.AluOpType.max, negate=True,
                    )
                    s_col = softp.tile([P, 1], F32, tag="s_col")
                    p_sb = softp.tile([P, L], F32, tag="p_sb")
                    p_wr = p_sb[:, :].rearrange("p (j c) -> p c j", c=2)
                    sim_rd = sim_ps[:, :].rearrange("p (c j) -> p c j", c=2)
                    nc.scalar.activation(
                        p_wr, sim_rd, mybir.ActivationFunctionType.Exp,
                        bias=neg_m, scale=1.0, accum_out=s_col,
                    )
                    # t = exp(theta - m): mask threshold in exp-space
                    t_col = softp.tile([P, 1], F32, tag="t_col")
                    nc.scalar.activation(
                        t_col, th_bc[:, g : g + 1],
                        mybir.ActivationFunctionType.Exp, bias=neg_m,
                    )
                    r_col = softp.tile([P, 1], F32, tag="r_col")
                    nc.vector.reciprocal(r_col, s_col)
                    msk = softp.tile([P, L], F32, tag="msk")
                    nc.vector.tensor_scalar(
                        out=msk, in0=p_sb,
                        scalar1=t_col, scalar2=r_col,
                        op0=mybir.AluOpType.is_gt, op1=mybir.AluOpType.mult,
                    )
                    # attn = p * msk (p already true-j ordered)
                    eng_mul.tensor_mul(
                        a_full[:, ci * L : (ci + 1) * L], p_sb, msk
                    )
                nc.sync.dma_start(
                    out=out[g].rearrange("p a b -> p (a b)"), in_=a_full
                )

            # ---- interleaved schedule ----
            for blk in range(NBLK):
                for gp in range(blk * BLK // 2, (blk + 1) * BLK // 2):
                    phase1_pair(gp)
                mlp_block(blk)
                if blk >= 1:
                    for g in range((blk - 1) * BLK, blk * BLK):
                        phase3_group(g)
            for g in range((NBLK - 1) * BLK, G):
                phase3_group(g)

    _split_multi_waits(nc)
    return nc


def _split_multi_waits(nc):
    """Compute-engine instruction structs encode a single sync-wait; move
    extra waits onto standalone NoOps in front of the instruction."""
    import dataclasses

    nid = [0]
    for f in nc.m.functions:
        for bb in f.blocks:
            new = []
            for inst in bb.instructions:
                si = inst.sync_info
                if si is not None and len(si.on_wait) > 1:
                    for w in si.on_wait[:-1]:
                        nid[0] += 1
                        nop = mybir.InstNoOp(
                            name=f"IWS-{nid[0]}",
                            engine=inst.engine,
                            ins=[],
                            outs=[],
                            sync_info=mybir.SyncInfo(on_wait=[w], on_update=[]),
                            bass_nofuse=True,
                        )
                        new.append(nop)
                    inst.sync_info = dataclasses.replace(
                        si, on_wait=[si.on_wait[-1]]
                    )
                new.append(inst)
            bb.instructions[:] = new


def _prep_consts(Wq, bq, Wk, bk, W1, W2a, W2b):
    wq_b = np.broadcast_to(Wq[0][None, :], (P, D)).astype(np.float32).copy()
    wk_b = np.broadcast_to(Wk[0][None, :], (P, D)).astype(np.float32).copy()
    w1c = np.stack([W1[0, 0::2], W1[0, 1::2]], axis=1).astype(np.float32).copy()
    w1re = np.broadcast_to(W1[0, 0::2][:, None], (P, P)).astype(np.float32).copy()
    w1ro = np.broadcast_to(W1[0, 1::2][:, None], (P, P)).astype(np.float32).copy()
    w2aT = np.ascontiguousarray(W2a.T)  # [i, o]
    w2aT_e = np.ascontiguousarray(w2aT[0::2, :])
    w2aT_o = np.ascontiguousarray(w2aT[1::2, :])
    w2bc = np.stack([W2b[0, 0:P], W2b[0, P : 2 * P]], axis=1).astype(np.float32).copy()
    ident = np.eye(P, dtype=np.float32)
    ones_row = np.ones((1, P), dtype=np.float32)
    return dict(
        wq_b=wq_b, wk_b=wk_b, w1c=w1c, w1re=w1re, w1ro=w1ro,
        w2aT_e=w2aT_e, w2aT_o=w2aT_o,
        w2bc=w2bc, ident=ident, ones_row=ones_row,
    )


def kernel(q, k, Wq, bq, Wk, bk, W1, W2a, W2b, _trace=False, _trace_kwargs=None):
    q = np.ascontiguousarray(np.asarray(q, dtype=np.float32))
    k = np.ascontiguousarray(np.asarray(k, dtype=np.float32))
    b, h, B, Lq, Dq = q.shape
    GT = b * h * B
    qf = q.reshape(GT, Lq, Dq)
    kf = k.reshape(GT, Lq, Dq)

    consts = _prep_consts(
        np.asarray(Wq), np.asarray(bq), np.asarray(Wk), np.asarray(bk),
        np.asarray(W1), np.asarray(W2a), np.asarray(W2b),
    )

    if "nc" not in _CACHE:
        _CACHE["nc"] = _build_program()
    nc = _CACHE["nc"]

    gpc = GT // N_CORES
    in_maps = []
    for c in range(N_CORES):
        qs = qf[c * gpc : (c + 1) * gpc].reshape(gpc, P, 2 * Dq)
        ks = kf[c * gpc : (c + 1) * gpc].reshape(gpc, P, 2 * Dq)
        m = {"q": np.ascontiguousarray(qs), "k": np.ascontiguousarray(ks)}
        m.update(consts)
        in_maps.append(m)

    kwargs = {}
    if _trace:
        kwargs["trace"] = True
        if _trace_kwargs:
            kwargs.update(_trace_kwargs)
    res = run_bass_kernel_spmd(nc, in_maps, core_ids=list(range(N_CORES)), **kwargs)
    outs = [r["attn"].reshape(gpc, Lq, Lq) for r in res.results]
    full = np.concatenate(outs, axis=0).reshape(b, h, B, Lq, Lq)
    if _trace:
        _CACHE["last_result"] = res
    return full

